# revision 19
# baseline (speedup 1.0000x reference)
"""Trainium2 Bass kernel for nn_Deep_AD_F_58213986730479 (dense_cnn).

Math (per iteration t of 3):
    feats = 4 one-pixel zero-padded shifts (N,S,W,E) of x        [n,4,h,w]
    d     = conv3x3(feats, W[t]) + b[t]                          [n,4,h,w]
    x    -= sum_k d_k * exp(-d_k^2) / 4

Implementation (v3, fp8 DoubleRow supersteps):
  - Pure data parallel: batch 32 -> 8 cores x 4 images.
  - The shift+conv composes into a 21-tap stencil: 5 column-banded matrices
    B_Dx (Dx=-2..2) applied at column shifts. On TRN2 the PE streams one
    OUTPUT column per cycle regardless of dtype, so fp8 DoubleRow (two
    (lhsT, shifted-rhs) band applications summed per instruction) is the
    only instruction-count lever: pairs (-2,-1), (0,+1), (-1res, ... ) wait
    (B-1_residual, B0) cover the 5 bands + one fp8 residual in 3 matmuls
    per (k, tile, image). Guard columns on the fp8 x copy let every pair
    run the full 512-col output range.
  - Supersteps (t, j): the 3 pair-lhsT per k are applied to all 4 images
    back-to-back (weight reuse), accumulating d_k [128, 4*512] in PSUM
    (4 banks, double-buffered per k).
  - Per-channel bias rides the Derivative_Erf activation (bias operand),
    one batched [128, 2048] ACT per (k): e = 2/sqrt(pi) exp(-(d+b)^2).
  - gated g_k = d_k * e_k on DVE (PSUM-fp32 read, bf16 out).
  - Sum tree + update in bf16 SBUF where plain TensorTensor has a fast
    2-byte path (~0.65 ns/col): s01/s23 on GpSimd, stot on DVE, update as
    (stot * masktile) then add-to-x, split DVE/GpSimd.
  - x lives in bf16 between iterations; Scalar converts it to fp8 for the
    next iteration's matmuls; the last iteration's update writes f32 into
    the staging tile which DMAs out.
  - 512 rows -> 5 row-tiles owning 103/103/103/103/100 rows with a 6-row
    halo each side (the valid region shrinks 2 rows/iter, so no cross-tile
    traffic); row-edge boundary fixes are baked into top/bot variants of
    B_{-1,0,1}; column-edge phantom corrections are 1-col fp8 matmuls.
"""
import sys

sys.path.insert(0, "/opt/trn_rl_repo")

import math
import numpy as np

import concourse.bass as bass
import concourse.bacc as bacc
import concourse.mybir as mybir
from concourse.ap import AP
from concourse.tile import TileContext
from concourse.bass_utils import run_bass_kernel_spmd


F32 = mybir.dt.float32
BF16 = mybir.dt.bfloat16
FP8 = mybir.dt.float8e4
AF = mybir.ActivationFunctionType
ALU = mybir.AluOpType
DR = mybir.MatmulPerfMode.DoubleRow

NCORES = 8
IMGS = 4
H = W_IMG = 512
T_ITERS = 3
KCH = 4
NTILES = 5
OWN = [103, 103, 103, 103, 100]      # owned rows per tile
OSTART = [0, 103, 206, 309, 412]     # first owned image row
RSTART = [-6, 97, 200, 303, 406]     # image row at partition 0
PLO = [6, 0, 0, 0, 0]                # first loaded partition
PHI = [128, 128, 128, 128, 106]      # end of loaded partitions
MLO = [6, 0, 0, 0, 0]                # update-mask range
MHI = [115, 115, 115, 115, 106]
OWN_P0 = 6                           # owned rows start here in every tile
C_UPD = math.sqrt(math.pi) / 8.0     # 1/4 * sqrt(pi)/2 (Derivative_Erf scale)
XQW = 520                            # fp8 x tile: 2 guard + 512 + guards
GL = 2                               # left guard cols
IMW = IMGS * W_IMG                   # 2048

# feats channel order in reference: N, S, W, E
OY = [-1, 1, 0, 0]
OX = [0, 0, -1, 1]

# weight tile layout per (t,k): 3 classes x 3 pairs x 256 + corrL/R = 2560
WTILE_COLS = 2560
# pair p rhs left-slot Dx (slots are (a, a+1))
PAIR_A = [-2, 1, -1]


def _composite_taps(Wc):
    taps = np.zeros((T_ITERS, KCH, 5, 5), np.float64)
    for t in range(T_ITERS):
        for k in range(KCH):
            for i in range(4):
                for dy in (-1, 0, 1):
                    for dx in (-1, 0, 1):
                        taps[t, k, dy + OY[i] + 2, dx + OX[i] + 2] += Wc[
                            t, k, i, dy + 1, dx + 1
                        ]
    return taps


def _band(vals_by_dy):
    B = np.zeros((128, 128), np.float64)
    for dy, v in vals_by_dy.items():
        B += v * np.eye(128, k=-dy)
    return B


def _build_wq(Wc):
    """fp8 weight image [128, 12*WTILE_COLS].

    Per (t,k), per class (top/mid/bot), three DoubleRow pairs; pair p's rhs
    slots read x at column shifts (a, a+1) with a = PAIR_A[p]:
      P1 a=-2: (B_{-2}, B_{-1})
      P2 a=+1: (B_{+1}, B_{+2})
      P3 a=-1: (fp8-residual of B_{-1}, B_0)
    then corrL [128], corrR [128] column-edge corrections.
    """
    taps = _composite_taps(Wc)
    npdt = mybir.dt.np(FP8)

    def q8(M):
        return M.astype(np.float32).astype(npdt).astype(np.float64)

    out = np.zeros((128, T_ITERS * KCH * WTILE_COLS), np.float64)
    for t in range(T_ITERS):
        for k in range(KCH):
            base = (t * KCH + k) * WTILE_COLS
            per_dx = {
                Dx: _band({Dy: taps[t, k, Dy + 2, Dx + 2] for Dy in range(-2, 3)})
                for Dx in (-2, -1, 0, 1, 2)
            }
            for ci, fix in enumerate((0, None, 1)):
                v = {}
                for Dx in (-2, -1, 0, 1, 2):
                    B = per_dx[Dx].copy()
                    if Dx in (-1, 0, 1):
                        if fix == 0:
                            B[OWN_P0, OWN_P0] -= Wc[t, k, 1, 0, Dx + 1]
                        elif fix == 1:
                            B[105, 105] -= Wc[t, k, 0, 2, Dx + 1]
                    v[Dx] = B
                r1 = v[-1] - q8(v[-1])  # fp8 residual of B_{-1}
                pairs = [
                    (v[-2], v[-1]),
                    (v[1], v[2]),
                    (r1, v[0]),
                ]
                for pi, (a_m, b_m) in enumerate(pairs):
                    off = base + (ci * 3 + pi) * 256
                    out[:, off : off + 128] = a_m
                    out[:, off + 128 : off + 256] = b_m
            out[:, base + 2304 : base + 2432] = _band(
                {dy: -Wc[t, k, 3, dy + 1, 0] for dy in (-1, 0, 1)}
            )
            out[:, base + 2432 : base + 2560] = _band(
                {dy: -Wc[t, k, 2, dy + 1, 2] for dy in (-1, 0, 1)}
            )
    return np.ascontiguousarray(out.astype(np.float32)).astype(npdt)


def _build_masks():
    import ml_dtypes

    m = np.zeros((128, NTILES * W_IMG), np.float32)
    for j in range(NTILES):
        m[MLO[j] : MHI[j], j * W_IMG : (j + 1) * W_IMG] = -C_UPD
    return m.astype(ml_dtypes.bfloat16)


def _build_nc(bvals):
    nc = bacc.Bacc(None, target_bir_lowering=False)
    xs = nc.declare_dram_parameter("xs", [IMGS, H, W_IMG], F32, isOutput=False)
    wq = nc.declare_dram_parameter(
        "wq", [128, T_ITERS * KCH * WTILE_COLS], FP8, isOutput=False
    )
    mk = nc.declare_dram_parameter("mk", [128, NTILES * W_IMG], BF16, isOutput=False)
    yo = nc.declare_dram_parameter("out", [IMGS, H, W_IMG], F32, isOutput=True)

    with TileContext(nc) as tc:
        with (
            tc.tile_pool(name="wts", bufs=1) as wp,
            tc.tile_pool(name="xdata", bufs=1) as xp,
            tc.tile_pool(name="work", bufs=2) as sp,
            tc.tile_pool(name="ps", bufs=2, space="PSUM") as pp,
        ):
            maskt = wp.tile([128, NTILES * W_IMG], BF16, tag="maskt")
            nc.sync.dma_start(out=maskt[:], in_=mk[:])
            bias_t = wp.tile([128, T_ITERS * KCH], F32, tag="bias")

            wt = {}
            for t in range(T_ITERS):
                for k in range(KCH):
                    wt[t, k] = wp.tile(
                        [128, WTILE_COLS], FP8, tag=f"wt{t}_{k}", name=f"wt{t}_{k}"
                    )

            def load_wt(t, k):
                off = (t * KCH + k) * WTILE_COLS
                nc.sync.dma_start(out=wt[t, k][:], in_=wq[:, off : off + WTILE_COLS])

            stage, xq = {}, {}
            xb = ({}, {})
            conv_eng = [nc.scalar, nc.gpsimd, nc.vector]
            for im in range(IMGS):
                for j in range(NTILES):
                    st = xp.tile(
                        [128, W_IMG], F32, tag=f"st{im}_{j}", name=f"st{im}_{j}"
                    )
                    stage[im, j] = st
                    if PLO[j] > 0:
                        nc.vector.memset(st[0 : PLO[j], :], 0.0)
                    if PHI[j] < 128:
                        p0 = (PHI[j] // 32) * 32
                        nc.vector.memset(st[p0:128, :], 0.0)
                    nc.sync.dma_start(
                        out=st[PLO[j] : PHI[j], :],
                        in_=xs[im, RSTART[j] + PLO[j] : RSTART[j] + PHI[j], :],
                    )
                    if im == 0 and j < KCH:
                        load_wt(0, j)
                if 1 <= im < T_ITERS:
                    for k in range(KCH):
                        load_wt(im, k)
                if im == 0:
                    # emitted after image-0 loads so the memsets don't gate
                    # the first x DMA on the vector queue
                    for t in range(T_ITERS):
                        for k in range(KCH):
                            c = t * KCH + k
                            nc.vector.memset(
                                bias_t[:, c : c + 1], float(bvals[t, k])
                            )

            for im in range(IMGS):
                for j in range(NTILES):
                    q = xp.tile([128, XQW], FP8, tag=f"xq{im}_{j}", name=f"xq{im}_{j}")
                    xq[im, j] = q
                    nc.vector.memset(q[:], 0.0)
                    for v in range(2):
                        xb[v][im, j] = xp.tile(
                            [128, W_IMG], BF16,
                            tag=f"xb{v}_{im}_{j}", name=f"xb{v}_{im}_{j}",
                        )
                    eng = conv_eng[(im * NTILES + j) % 3]
                    if eng is nc.scalar:
                        eng.copy(q[:, GL : GL + W_IMG], stage[im, j][:])
                    else:
                        eng.tensor_copy(
                            out=q[:, GL : GL + W_IMG], in_=stage[im, j][:]
                        )

            lhs_dims = [[WTILE_COLS, 128], [128, 2], [1, 128]]
            HW2 = 2 * W_IMG  # half-superstep width (2 images)

            pending_tail = None
            for t in range(T_ITERS):
                for j in range(NTILES):
                    cls = 0 if j == 0 else (2 if j == NTILES - 1 else 1)
                    gk = []
                    for k in range(KCH):
                        c = t * KCH + k
                        w = wt[t, k]
                        wh, woff = w[:].tensor, w[:].offset
                        g = sp.tile([128, IMW], BF16, tag=f"g{k}")
                        # two PSUM halves (2 images each) so ACT/gated can
                        # drain half k while the PE fills the other half
                        for h in range(2):
                            d = pp.tile([128, HW2], F32, tag=f"d{h}", name=f"d{h}")
                            for pi in range(3):
                                wcol = (cls * 3 + pi) * 256
                                lhs = AP(wh, woff + wcol, lhs_dims)
                                a = PAIR_A[pi]
                                for hi, im in enumerate((2 * h, 2 * h + 1)):
                                    q = xq[im, j]
                                    qh, qoff = q[:].tensor, q[:].offset
                                    rhs = AP(
                                        qh,
                                        qoff + GL + a,
                                        [[XQW, 128], [1, 2], [1, W_IMG]],
                                    )
                                    nc.tensor.matmul(
                                        d[:, hi * W_IMG : (hi + 1) * W_IMG],
                                        lhs,
                                        rhs,
                                        start=(pi == 0),
                                        stop=False,
                                        perf_mode=DR,
                                    )
                            for hi, im in enumerate((2 * h, 2 * h + 1)):
                                nc.tensor.matmul(
                                    d[:, hi * W_IMG : hi * W_IMG + 1],
                                    w[:, 2304:2432],
                                    xq[im, j][:, GL : GL + 1],
                                    start=False,
                                    stop=False,
                                )
                            for hi, im in enumerate((2 * h, 2 * h + 1)):
                                nc.tensor.matmul(
                                    d[:, (hi + 1) * W_IMG - 1 : (hi + 1) * W_IMG],
                                    w[:, 2432:2560],
                                    xq[im, j][:, GL + W_IMG - 1 : GL + W_IMG],
                                    start=False,
                                    stop=(hi == 1),
                                )
                            e = sp.tile([128, HW2], BF16, tag=f"e{h}")
                            nc.scalar.activation(
                                e[:], d[:], AF.Derivative_Erf,
                                bias=bias_t[:, c : c + 1],
                            )
                            nc.vector.scalar_tensor_tensor(
                                out=g[:, h * HW2 : (h + 1) * HW2],
                                in0=d[:],
                                scalar=bias_t[:, c : c + 1],
                                in1=e[:],
                                op0=ALU.add,
                                op1=ALU.mult,
                            )
                        gk.append(g)

                    def make_tail(t, j, gk):
                        def tail():
                            s01 = sp.tile([128, IMW], BF16, tag="s01")
                            nc.gpsimd.tensor_tensor(
                                out=s01[:], in0=gk[0][:], in1=gk[1][:], op=ALU.add
                            )
                            s23 = sp.tile([128, IMW], BF16, tag="s23")
                            nc.gpsimd.tensor_tensor(
                                out=s23[:], in0=gk[2][:], in1=gk[3][:], op=ALU.add
                            )
                            stot = sp.tile([128, IMW], BF16, tag="stot")
                            nc.vector.tensor_tensor(
                                out=stot[:], in0=s01[:], in1=s23[:], op=ALU.add
                            )
                            mask = maskt[:, j * W_IMG : (j + 1) * W_IMG]
                            tmps = []
                            for im in range(IMGS):
                                blk = stot[:, im * W_IMG : (im + 1) * W_IMG]
                                tmp = sp.tile([128, W_IMG], BF16, tag=f"tmp{im % 2}")
                                # bf16 multiply on DVE hits a slow path; GpSimd
                                # runs it at its normal rate
                                nc.gpsimd.tensor_tensor(
                                    out=tmp[:], in0=blk, in1=mask, op=ALU.mult
                                )
                                tmps.append(tmp)
                            for im in range(IMGS):
                                tmp = tmps[im]
                                if t == 0:
                                    nc.vector.tensor_tensor(
                                        out=xb[0][im, j][:],
                                        in0=tmp[:],
                                        in1=stage[im, j][:],
                                        op=ALU.add,
                                    )
                                elif t == T_ITERS - 1:
                                    nc.vector.tensor_tensor(
                                        out=stage[im, j][:],
                                        in0=tmp[:],
                                        in1=xb[1][im, j][:],
                                        op=ALU.add,
                                    )
                                    nc.sync.dma_start(
                                        out=yo[im, OSTART[j] : OSTART[j] + OWN[j], :],
                                        in_=stage[im, j][OWN_P0 : OWN_P0 + OWN[j], :],
                                    )
                                else:
                                    nc.vector.tensor_tensor(
                                        out=xb[1][im, j][:],
                                        in0=tmp[:],
                                        in1=xb[0][im, j][:],
                                        op=ALU.add,
                                    )
                                if t < T_ITERS - 1:
                                    nc.scalar.copy(
                                        xq[im, j][:, GL : GL + W_IMG], xb[t][im, j][:]
                                    )
                        return tail

                    # software pipelining: emit the previous superstep's
                    # sum/update tail AFTER this superstep's gated ops so
                    # per-engine FIFOs never head-of-line block
                    if pending_tail is not None:
                        pending_tail()
                    pending_tail = make_tail(t, j, gk)
            if pending_tail is not None:
                pending_tail()
    nc.compile()
    return nc


_CACHE = {}


def _get_program(Wc, bc):
    key = (Wc.tobytes(), bc.tobytes())
    if key not in _CACHE:
        wqarr = _build_wq(Wc.astype(np.float64))
        nc = _build_nc(bc.astype(np.float64))
        _CACHE[key] = (nc, wqarr, _build_masks())
    return _CACHE[key]


def _install_trace_shim():
    """The agent image lacks antenv.axon_hooks; rebuild the NTFF hook from
    trn_boot's ctypes recipe and skip the artifact upload."""
    import types

    if "antenv.axon_hooks" in sys.modules:
        return
    try:
        from trn_agent_boot.trn_boot import _ntff_profile_via_ctypes

        hook = _ntff_profile_via_ctypes("/opt/axon/libaxon_pjrt.so")
    except Exception:
        hook = None
    mod = types.ModuleType("antenv.axon_hooks")
    mod.get_axon_ntff_profile_hook = lambda: hook
    mod.set_axon_ntff_profile_hook = lambda h: None
    sys.modules["antenv.axon_hooks"] = mod
    import concourse.bass_utils as bu

    bu.upload_artifacts = lambda d: "local://skipped"


def kernel(x, W, b, _trace=False, _tracedir=None):
    x = np.asarray(x)
    W = np.asarray(W)
    b = np.asarray(b)
    nc, wqarr, mkarr = _get_program(W, b)
    in_maps = []
    for c in range(NCORES):
        shard = np.ascontiguousarray(x[c * IMGS : (c + 1) * IMGS, 0]).astype(np.float32)
        in_maps.append({"xs": shard, "wq": wqarr, "mk": mkarr})
    kw = {}
    if _trace:
        _install_trace_shim()
        kw = {"trace": True, "tmpdir": _tracedir}
    res = run_bass_kernel_spmd(nc, in_maps, list(range(NCORES)), **kw)
    out = np.concatenate([res.results[c]["out"] for c in range(NCORES)], axis=0)
    out = out[:, None].astype(x.dtype)
    kernel._last = res
    return out


# revision 21
# speedup vs baseline: 1.0285x; 1.0285x over previous
"""Trainium2 Bass kernel for nn_Deep_AD_F_58213986730479 (dense_cnn).

Math (per iteration t of 3):
    feats = 4 one-pixel zero-padded shifts (N,S,W,E) of x        [n,4,h,w]
    d     = conv3x3(feats, W[t]) + b[t]                          [n,4,h,w]
    x    -= sum_k d_k * exp(-d_k^2) / 4

Implementation (v3, fp8 DoubleRow supersteps):
  - Pure data parallel: batch 32 -> 8 cores x 4 images.
  - The shift+conv composes into a 21-tap stencil: 5 column-banded matrices
    B_Dx (Dx=-2..2) applied at column shifts. On TRN2 the PE streams one
    OUTPUT column per cycle regardless of dtype, so fp8 DoubleRow (two
    (lhsT, shifted-rhs) band applications summed per instruction) is the
    only instruction-count lever: pairs (-2,-1), (0,+1), (-1res, ... ) wait
    (B-1_residual, B0) cover the 5 bands + one fp8 residual in 3 matmuls
    per (k, tile, image). Guard columns on the fp8 x copy let every pair
    run the full 512-col output range.
  - Supersteps (t, j): the 3 pair-lhsT per k are applied to all 4 images
    back-to-back (weight reuse), accumulating d_k [128, 4*512] in PSUM
    (4 banks, double-buffered per k).
  - Per-channel bias rides the Derivative_Erf activation (bias operand),
    one batched [128, 2048] ACT per (k): e = 2/sqrt(pi) exp(-(d+b)^2).
  - gated g_k = d_k * e_k on DVE (PSUM-fp32 read, bf16 out).
  - Sum tree + update in bf16 SBUF where plain TensorTensor has a fast
    2-byte path (~0.65 ns/col): s01/s23 on GpSimd, stot on DVE, update as
    (stot * masktile) then add-to-x, split DVE/GpSimd.
  - x lives in bf16 between iterations; Scalar converts it to fp8 for the
    next iteration's matmuls; the last iteration's update writes f32 into
    the staging tile which DMAs out.
  - 512 rows -> 5 row-tiles owning 103/103/103/103/100 rows with a 6-row
    halo each side (the valid region shrinks 2 rows/iter, so no cross-tile
    traffic); row-edge boundary fixes are baked into top/bot variants of
    B_{-1,0,1}; column-edge phantom corrections are 1-col fp8 matmuls.
"""
import sys

sys.path.insert(0, "/opt/trn_rl_repo")

import math
import numpy as np

import concourse.bass as bass
import concourse.bacc as bacc
import concourse.mybir as mybir
from concourse.ap import AP
from concourse.tile import TileContext
from concourse.bass_utils import run_bass_kernel_spmd


F32 = mybir.dt.float32
BF16 = mybir.dt.bfloat16
FP8 = mybir.dt.float8e4
AF = mybir.ActivationFunctionType
ALU = mybir.AluOpType
DR = mybir.MatmulPerfMode.DoubleRow

NCORES = 8
IMGS = 4
H = W_IMG = 512
T_ITERS = 3
KCH = 4
NTILES = 5
OWN = [103, 103, 103, 103, 100]      # owned rows per tile
OSTART = [0, 103, 206, 309, 412]     # first owned image row
RSTART = [-6, 97, 200, 303, 406]     # image row at partition 0
PLO = [6, 0, 0, 0, 0]                # first loaded partition
PHI = [128, 128, 128, 128, 106]      # end of loaded partitions
MLO = [6, 0, 0, 0, 0]                # update-mask range
MHI = [115, 115, 115, 115, 106]
OWN_P0 = 6                           # owned rows start here in every tile
C_UPD = math.sqrt(math.pi) / 8.0     # 1/4 * sqrt(pi)/2 (Derivative_Erf scale)
XQW = 520                            # fp8 x tile: 2 guard + 512 + guards
GL = 2                               # left guard cols
IMW = IMGS * W_IMG                   # 2048

# feats channel order in reference: N, S, W, E
OY = [-1, 1, 0, 0]
OX = [0, 0, -1, 1]

# weight tile layout per (t,k): 3 classes x 3 pairs x 256 + corrL/R = 2560
WTILE_COLS = 2560
# pair p rhs left-slot Dx (slots are (a, a+1))
PAIR_A = [-2, 1, -1]


def _composite_taps(Wc):
    taps = np.zeros((T_ITERS, KCH, 5, 5), np.float64)
    for t in range(T_ITERS):
        for k in range(KCH):
            for i in range(4):
                for dy in (-1, 0, 1):
                    for dx in (-1, 0, 1):
                        taps[t, k, dy + OY[i] + 2, dx + OX[i] + 2] += Wc[
                            t, k, i, dy + 1, dx + 1
                        ]
    return taps


def _band(vals_by_dy):
    B = np.zeros((128, 128), np.float64)
    for dy, v in vals_by_dy.items():
        B += v * np.eye(128, k=-dy)
    return B


def _build_wq(Wc):
    """fp8 weight image [128, 12*WTILE_COLS].

    Per (t,k), per class (top/mid/bot), three DoubleRow pairs; pair p's rhs
    slots read x at column shifts (a, a+1) with a = PAIR_A[p]:
      P1 a=-2: (B_{-2}, B_{-1})
      P2 a=+1: (B_{+1}, B_{+2})
      P3 a=-1: (fp8-residual of B_{-1}, B_0)
    then corrL [128], corrR [128] column-edge corrections.
    """
    taps = _composite_taps(Wc)
    npdt = mybir.dt.np(FP8)

    def q8(M):
        return M.astype(np.float32).astype(npdt).astype(np.float64)

    out = np.zeros((128, T_ITERS * KCH * WTILE_COLS), np.float64)
    for t in range(T_ITERS):
        for k in range(KCH):
            base = (t * KCH + k) * WTILE_COLS
            per_dx = {
                Dx: _band({Dy: taps[t, k, Dy + 2, Dx + 2] for Dy in range(-2, 3)})
                for Dx in (-2, -1, 0, 1, 2)
            }
            for ci, fix in enumerate((0, None, 1)):
                v = {}
                for Dx in (-2, -1, 0, 1, 2):
                    B = per_dx[Dx].copy()
                    if Dx in (-1, 0, 1):
                        if fix == 0:
                            B[OWN_P0, OWN_P0] -= Wc[t, k, 1, 0, Dx + 1]
                        elif fix == 1:
                            B[105, 105] -= Wc[t, k, 0, 2, Dx + 1]
                    v[Dx] = B
                r1 = v[-1] - q8(v[-1])  # fp8 residual of B_{-1}
                pairs = [
                    (v[-2], v[-1]),
                    (v[1], v[2]),
                    (r1, v[0]),
                ]
                for pi, (a_m, b_m) in enumerate(pairs):
                    off = base + (ci * 3 + pi) * 256
                    out[:, off : off + 128] = a_m
                    out[:, off + 128 : off + 256] = b_m
            out[:, base + 2304 : base + 2432] = _band(
                {dy: -Wc[t, k, 3, dy + 1, 0] for dy in (-1, 0, 1)}
            )
            out[:, base + 2432 : base + 2560] = _band(
                {dy: -Wc[t, k, 2, dy + 1, 2] for dy in (-1, 0, 1)}
            )
    return np.ascontiguousarray(out.astype(np.float32)).astype(npdt)


def _build_masks():
    import ml_dtypes

    m = np.zeros((128, NTILES * W_IMG), np.float32)
    for j in range(NTILES):
        m[MLO[j] : MHI[j], j * W_IMG : (j + 1) * W_IMG] = -C_UPD
    return m.astype(ml_dtypes.bfloat16)


def _build_nc(bvals):
    nc = bacc.Bacc(None, target_bir_lowering=False)
    xs = nc.declare_dram_parameter("xs", [IMGS, H, W_IMG], F32, isOutput=False)
    wq = nc.declare_dram_parameter(
        "wq", [128, T_ITERS * KCH * WTILE_COLS], FP8, isOutput=False
    )
    mk = nc.declare_dram_parameter("mk", [128, NTILES * W_IMG], BF16, isOutput=False)
    yo = nc.declare_dram_parameter("out", [IMGS, H, W_IMG], F32, isOutput=True)

    with TileContext(nc) as tc:
        with (
            tc.tile_pool(name="wts", bufs=1) as wp,
            tc.tile_pool(name="xdata", bufs=1) as xp,
            tc.tile_pool(name="work", bufs=2) as sp,
            tc.tile_pool(name="ps", bufs=2, space="PSUM") as pp,
        ):
            maskt = wp.tile([128, NTILES * W_IMG], BF16, tag="maskt")
            nc.sync.dma_start(out=maskt[:], in_=mk[:])
            bias_t = wp.tile([128, T_ITERS * KCH], F32, tag="bias")

            wt = {}
            for t in range(T_ITERS):
                for k in range(KCH):
                    wt[t, k] = wp.tile(
                        [128, WTILE_COLS], FP8, tag=f"wt{t}_{k}", name=f"wt{t}_{k}"
                    )

            def load_wt(t, k):
                off = (t * KCH + k) * WTILE_COLS
                nc.sync.dma_start(out=wt[t, k][:], in_=wq[:, off : off + WTILE_COLS])

            stage, xq = {}, {}
            xb = ({}, {})
            for im in range(IMGS):
                for j in range(NTILES):
                    st = xp.tile(
                        [128, W_IMG], F32, tag=f"st{im}_{j}", name=f"st{im}_{j}"
                    )
                    stage[im, j] = st
                    if PLO[j] > 0:
                        nc.vector.memset(st[0 : PLO[j], :], 0.0)
                    if PHI[j] < 128:
                        p0 = (PHI[j] // 32) * 32
                        nc.vector.memset(st[p0:128, :], 0.0)
                    nc.sync.dma_start(
                        out=st[PLO[j] : PHI[j], :],
                        in_=xs[im, RSTART[j] + PLO[j] : RSTART[j] + PHI[j], :],
                    )
                    if im == 0 and j < KCH:
                        load_wt(0, j)
                if 1 <= im < T_ITERS:
                    for k in range(KCH):
                        load_wt(im, k)
                if im == 0:
                    for t in range(T_ITERS):
                        for k in range(KCH):
                            c = t * KCH + k
                            nc.vector.memset(
                                bias_t[:, c : c + 1], float(bvals[t, k])
                            )

            # one fp8 tile per row-tile j holding all 4 images side by side
            # (lets the edge-correction matmuls batch across images); only
            # the guard columns are memset -- the data blocks are fully
            # written by the conversions
            for j in range(NTILES):
                q = xp.tile([128, IMGS * XQW], FP8, tag=f"xq{j}", name=f"xq{j}")
                xq[j] = q
                nc.vector.memset(q[:, 0:GL], 0.0)
                for im in range(IMGS):
                    lo = im * XQW + GL + W_IMG
                    hi = min((im + 1) * XQW + GL, IMGS * XQW)
                    nc.vector.memset(q[:, lo:hi], 0.0)
                for im in range(IMGS):
                    xb[0][im, j] = xp.tile(
                        [128, W_IMG], BF16, tag=f"xb0_{im}_{j}", name=f"xb0_{im}_{j}"
                    )
                    xb[1][im, j] = xp.tile(
                        [128, W_IMG], BF16, tag=f"xb1_{im}_{j}", name=f"xb1_{im}_{j}"
                    )
            conv_eng = [nc.scalar, nc.gpsimd, nc.vector]
            for j in range(NTILES):
                for im in range(IMGS):
                    q = xq[j]
                    blk = q[:, im * XQW + GL : im * XQW + GL + W_IMG]
                    eng = conv_eng[(j * IMGS + im) % 3]
                    if eng is nc.scalar:
                        eng.copy(blk, stage[im, j][:])
                    else:
                        eng.tensor_copy(out=blk, in_=stage[im, j][:])

            lhs_dims = [[WTILE_COLS, 128], [128, 2], [1, 128]]
            HW2 = 2 * W_IMG  # half-superstep width (2 images)

            def emit_tail_part(part, st_):
                """Emit one chunk of the previous superstep's tail.
                part 0: stot + mask-mults; 1..3: update adds (+conv/dma)."""
                t, j, s01, s23, ctx = st_
                if part == 0:
                    stot = sp.tile([128, IMW], BF16, tag="stot")
                    nc.vector.tensor_tensor(
                        out=stot[:], in0=s01[:], in1=s23[:], op=ALU.add
                    )
                    mask = maskt[:, j * W_IMG : (j + 1) * W_IMG]
                    for im in range(IMGS):
                        blk = stot[:, im * W_IMG : (im + 1) * W_IMG]
                        tmp = sp.tile([128, W_IMG], BF16, tag=f"tmp{im % 2}")
                        nc.gpsimd.tensor_tensor(
                            out=tmp[:], in0=blk, in1=mask, op=ALU.mult
                        )
                        ctx.append(tmp)
                    return
                # parts 1..3 -> images [0], [1], [2,3]
                ims = {1: (0,), 2: (1,), 3: (2, 3)}[part]
                for im in ims:
                    tmp = ctx[im]
                    if t == 0:
                        nc.vector.tensor_tensor(
                            out=xb[0][im, j][:], in0=tmp[:],
                            in1=stage[im, j][:], op=ALU.add,
                        )
                    elif t == T_ITERS - 1:
                        nc.vector.tensor_tensor(
                            out=stage[im, j][:], in0=tmp[:],
                            in1=xb[1][im, j][:], op=ALU.add,
                        )
                        nc.sync.dma_start(
                            out=yo[im, OSTART[j] : OSTART[j] + OWN[j], :],
                            in_=stage[im, j][OWN_P0 : OWN_P0 + OWN[j], :],
                        )
                    else:
                        nc.vector.tensor_tensor(
                            out=xb[1][im, j][:], in0=tmp[:],
                            in1=xb[0][im, j][:], op=ALU.add,
                        )
                    if t < T_ITERS - 1:
                        nc.scalar.copy(
                            xq[j][:, im * XQW + GL : im * XQW + GL + W_IMG],
                            xb[t][im, j][:],
                        )

            pending = None
            for t in range(T_ITERS):
                for j in range(NTILES):
                    cls = 0 if j == 0 else (2 if j == NTILES - 1 else 1)
                    q = xq[j]
                    qh, qoff = q[:].tensor, q[:].offset
                    gk = []
                    for k in range(KCH):
                        c = t * KCH + k
                        w = wt[t, k]
                        wh, woff = w[:].tensor, w[:].offset
                        g = sp.tile([128, IMW], BF16, tag=f"g{k}")
                        for h in range(2):
                            d = pp.tile([128, HW2], F32, tag=f"d{h}", name=f"d{h}")
                            for pi in range(3):
                                wcol = (cls * 3 + pi) * 256
                                lhs = AP(wh, woff + wcol, lhs_dims)
                                a = PAIR_A[pi]
                                for hi, im in enumerate((2 * h, 2 * h + 1)):
                                    rhs = AP(
                                        qh,
                                        qoff + im * XQW + GL + a,
                                        [[IMGS * XQW, 128], [1, 2], [1, W_IMG]],
                                    )
                                    nc.tensor.matmul(
                                        d[:, hi * W_IMG : (hi + 1) * W_IMG],
                                        lhs,
                                        rhs,
                                        start=(pi == 0),
                                        stop=False,
                                        perf_mode=DR,
                                    )
                            # edge corrections, one inst per side covering
                            # both images of the half (strided out/rhs)
                            dh, doff = d[:].tensor, d[:].offset
                            for side, wlo, col in (
                                (0, 2304, GL),
                                (1, 2432, GL + W_IMG - 1),
                            ):
                                ocol = 0 if side == 0 else W_IMG - 1
                                out_ap = AP(dh, doff + ocol, [[HW2, 128], [W_IMG, 2]])
                                rhs_ap = AP(
                                    qh, qoff + 2 * h * XQW + col,
                                    [[IMGS * XQW, 128], [XQW, 2]],
                                )
                                nc.tensor.matmul(
                                    out_ap,
                                    w[:, wlo : wlo + 128],
                                    rhs_ap,
                                    start=False,
                                    stop=(side == 1),
                                )
                            e = sp.tile([128, HW2], BF16, tag=f"e{h}")
                            nc.scalar.activation(
                                e[:], d[:], AF.Derivative_Erf,
                                bias=bias_t[:, c : c + 1],
                            )
                            nc.vector.scalar_tensor_tensor(
                                out=g[:, h * HW2 : (h + 1) * HW2],
                                in0=d[:],
                                scalar=bias_t[:, c : c + 1],
                                in1=e[:],
                                op0=ALU.add,
                                op1=ALU.mult,
                            )
                        gk.append(g)
                        # interleave the previous superstep's tail between
                        # this superstep's k blocks so no engine FIFO ever
                        # head-of-line blocks the gated stream
                        if pending is not None:
                            emit_tail_part(k, pending)
                        if k == 1:
                            s01 = sp.tile([128, IMW], BF16, tag="s01")
                            nc.gpsimd.tensor_tensor(
                                out=s01[:], in0=gk[0][:], in1=gk[1][:], op=ALU.add
                            )
                        if k == 3:
                            s23 = sp.tile([128, IMW], BF16, tag="s23")
                            nc.gpsimd.tensor_tensor(
                                out=s23[:], in0=gk[2][:], in1=gk[3][:], op=ALU.add
                            )
                    pending = (t, j, s01, s23, [])
            if pending is not None:
                for part in range(4):
                    emit_tail_part(part, pending)
    nc.compile()
    return nc


_CACHE = {}


def _get_program(Wc, bc):
    key = (Wc.tobytes(), bc.tobytes())
    if key not in _CACHE:
        wqarr = _build_wq(Wc.astype(np.float64))
        nc = _build_nc(bc.astype(np.float64))
        _CACHE[key] = (nc, wqarr, _build_masks())
    return _CACHE[key]


def _install_trace_shim():
    """The agent image lacks antenv.axon_hooks; rebuild the NTFF hook from
    trn_boot's ctypes recipe and skip the artifact upload."""
    import types

    if "antenv.axon_hooks" in sys.modules:
        return
    try:
        from trn_agent_boot.trn_boot import _ntff_profile_via_ctypes

        hook = _ntff_profile_via_ctypes("/opt/axon/libaxon_pjrt.so")
    except Exception:
        hook = None
    mod = types.ModuleType("antenv.axon_hooks")
    mod.get_axon_ntff_profile_hook = lambda: hook
    mod.set_axon_ntff_profile_hook = lambda h: None
    sys.modules["antenv.axon_hooks"] = mod
    import concourse.bass_utils as bu

    bu.upload_artifacts = lambda d: "local://skipped"


def kernel(x, W, b, _trace=False, _tracedir=None):
    x = np.asarray(x)
    W = np.asarray(W)
    b = np.asarray(b)
    nc, wqarr, mkarr = _get_program(W, b)
    in_maps = []
    for c in range(NCORES):
        shard = np.ascontiguousarray(x[c * IMGS : (c + 1) * IMGS, 0]).astype(np.float32)
        in_maps.append({"xs": shard, "wq": wqarr, "mk": mkarr})
    kw = {}
    if _trace:
        _install_trace_shim()
        kw = {"trace": True, "tmpdir": _tracedir}
    res = run_bass_kernel_spmd(nc, in_maps, list(range(NCORES)), **kw)
    out = np.concatenate([res.results[c]["out"] for c in range(NCORES)], axis=0)
    out = out[:, None].astype(x.dtype)
    kernel._last = res
    return out


# revision 23
# speedup vs baseline: 1.2580x; 1.2231x over previous
"""Trainium2 Bass kernel for nn_Deep_AD_F_58213986730479 (dense_cnn).

Math (per iteration t of 3):
    feats = 4 one-pixel zero-padded shifts (N,S,W,E) of x        [n,4,h,w]
    d     = conv3x3(feats, W[t]) + b[t]                          [n,4,h,w]
    x    -= sum_k d_k * exp(-d_k^2) / 4

Implementation:
  - Pure data parallel: batch 32 -> 8 cores x 4 images.
  - The shift+conv composes into a 21-tap stencil on x. Vertical taps are
    applied with banded-matrix matmuls on TensorE (contraction over image
    rows on partitions); horizontal taps via 5 column-shifted accumulating
    matmuls into PSUM. Boundary semantics of the double zero-padding are
    exact: row-edge terms fold into per-tile band-matrix variants; column
    edge terms are two N=1 correction matmuls per channel.
  - Matmuls default to bf16 (1 cycle/row on the PE vs 4 for fp32); x stays
    f32 in SBUF and is converted per-tile by a ScalarE copy each iteration,
    so only the stencil inputs are rounded (rel err ~7e-4, gate is 2e-2).
  - exp(-d^2) comes from one ScalarE op: Derivative_Erf = 2/sqrt(pi)*exp(-x^2);
    the bias add (d+b) rides free in the activation and in the DVE
    scalar_tensor_tensor that forms gated = (d+b)*e. Channel sum on GpSimd,
    final x update is one fused DVE scalar_tensor_tensor.
  - Each 512x512 image is 5 row-tiles [128,512] (stride 116, 6-row halo);
    3 iterations shrink the valid halo by 2 rows each, so no cross-tile
    traffic is ever needed.
  - Startup critical path: weights are split into per-(t,k) SBUF tiles
    (dep tracking is tile-granular) and their DMAs are interleaved after
    image-0's x tiles so the first matmuls start ~10us earlier.
"""
import sys

sys.path.insert(0, "/opt/trn_rl_repo")

import math
import numpy as np

import concourse.bass as bass
import concourse.bacc as bacc
import concourse.mybir as mybir
from concourse.tile import TileContext
from concourse.bass_utils import run_bass_kernel_spmd

F32 = mybir.dt.float32
F32R = mybir.dt.float32r
BF16 = mybir.dt.bfloat16
AF = mybir.ActivationFunctionType
ALU = mybir.AluOpType

NCORES = 8
IMGS = 4          # images per core
H = W_IMG = 512
T_ITERS = 3
KCH = 4
NTILES = 5
TSTART = [-6, 110, 226, 342, 458]   # image row held by partition 0 of tile j
CORE_LO = 6                          # first owned partition of each tile
CORE_ROWS = [116, 116, 116, 116, 48]
C_UPD = math.sqrt(math.pi) / 8.0     # 1/4 * sqrt(pi)/2 (Derivative_Erf scale)

# feats channel order in reference: N, S, W, E
OY = [-1, 1, 0, 0]
OX = [0, 0, -1, 1]

DXS = [0, -1, 1, -2, 2]

# debug bisect flags
_SKIP_CORR = False
_BATCH_CORR = False  # batch edge-corrections: 5-col matmuls per (t,im,k,side)
_PSUM_BUFS = 2
_INPLACE_UPD = True
_MASK_AP = True
_TILE_SET = None  # e.g. [2] to restrict tiles (debug)
_PAD_BMAT = True
_MM_DTYPE = __import__("os").environ.get("KERNEL_MM_DTYPE", "bf16")  # f32 | f32r | bf16
_BATCH_CORR = _BATCH_CORR and _MM_DTYPE == "bf16"
_SIMPLE_BIAS = False  # Dx=0 first: full-range start=True write


def _composite_taps(Wc):
    """T[t,k,Dy+2,Dx+2] = sum of W[t,k,i,dy+1,dx+1] with dy+oy_i=Dy, dx+ox_i=Dx."""
    taps = np.zeros((T_ITERS, KCH, 5, 5), np.float64)
    for t in range(T_ITERS):
        for k in range(KCH):
            for i in range(4):
                for dy in (-1, 0, 1):
                    for dx in (-1, 0, 1):
                        taps[t, k, dy + OY[i] + 2, dx + OX[i] + 2] += Wc[
                            t, k, i, dy + 1, dx + 1
                        ]
    return taps


def _build_bmats(Wc):
    """Dense lhsT matrices, returned as array [NB,128,128] f32 plus an index fn.

    Layout per (t,k): 5 generic B_Dx, then 3 top-variant (Dx=-1,0,1), then
    3 bottom-variant, then left corr, right corr = 13 matrices.
    B[in_row, out_row] = tap[in-out, Dx].
    """
    taps = _composite_taps(Wc)
    mats = []
    index = {}

    def band(vals_by_dy):
        B = np.zeros((128, 128), np.float64)
        for dy, v in vals_by_dy.items():
            B += v * np.eye(128, k=-dy)
        return B

    for t in range(T_ITERS):
        for k in range(KCH):
            per_dx = {}
            for Dx in (-2, -1, 0, 1, 2):
                per_dx[Dx] = band(
                    {Dy: taps[t, k, Dy + 2, Dx + 2] for Dy in range(-2, 3)}
                )
            for Dx in DXS:
                index[(t, k, Dx, "mid")] = len(mats)
                mats.append(per_dx[Dx])
            for Dx in (-1, 0, 1):
                Btop = per_dx[Dx].copy()
                # image row 0 = partition CORE_LO of tile 0: remove south-ch dy=-1
                Btop[CORE_LO, CORE_LO] -= Wc[t, k, 1, 0, Dx + 1]
                index[(t, k, Dx, "top")] = len(mats)
                mats.append(Btop)
            for Dx in (-1, 0, 1):
                Bbot = per_dx[Dx].copy()
                # image row 511 = partition 53 of tile 4: remove north-ch dy=+1
                p = CORE_LO + (H - 1) - TSTART[4] - CORE_LO  # = 53
                Bbot[p, p] -= Wc[t, k, 0, 2, Dx + 1]
                index[(t, k, Dx, "bot")] = len(mats)
                mats.append(Bbot)
            # column-edge corrections (vertical 3-tap bands)
            Bl = band({dy: -Wc[t, k, 3, dy + 1, 0] for dy in (-1, 0, 1)})
            index[(t, k, "corrL")] = len(mats)
            mats.append(Bl)
            Br = band({dy: -Wc[t, k, 2, dy + 1, 2] for dy in (-1, 0, 1)})
            index[(t, k, "corrR")] = len(mats)
            mats.append(Br)
    arr = np.stack(mats).astype(np.float32)
    return arr, index


_NB = T_ITERS * KCH * 13


def _build_masks():
    """Per-tile update masks [128, NTILES]: -C_UPD at real image rows, 0 at pad."""
    m = np.full((128, NTILES), -C_UPD, np.float32)
    for j in range(NTILES):
        r0 = TSTART[j]
        plo = max(0, -r0)
        phi = min(128, H - r0)
        m[0:plo, j] = 0.0
        m[phi:128, j] = 0.0
    return m


def _build_nc(bvals, bindex):
    nc = bacc.Bacc(None, target_bir_lowering=False)
    xs = nc.declare_dram_parameter("xs", [IMGS, H, W_IMG], F32, isOutput=False)
    bmw = _NB * 128 + (0 if _MM_DTYPE == "bf16" else NTILES)
    if _PAD_BMAT:
        bmw = (bmw + 127) // 128 * 128
    bdt = BF16 if _MM_DTYPE == "bf16" else F32
    bm = nc.declare_dram_parameter("bmat", [128, bmw], bdt, isOutput=False)
    if _MM_DTYPE == "bf16":
        aux = nc.declare_dram_parameter("aux", [128, 16], F32, isOutput=False)
    yo = nc.declare_dram_parameter("out", [IMGS, H, W_IMG], F32, isOutput=True)

    with TileContext(nc) as tc:
        with (
            tc.tile_pool(name="wts", bufs=1) as wp,
            tc.tile_pool(name="xdata", bufs=1) as xp,
            tc.tile_pool(name="work", bufs=4) as sp,
            tc.tile_pool(name="ps", bufs=2, space="PSUM") as pp,
            tc.tile_pool(name="ps1", bufs=1, space="PSUM") as pp1,
            tc.tile_pool(name="psc", bufs=1, space="PSUM") as ppc,
        ):
            pert = KCH * 13 * 128  # cols per iteration t
            perk = 13 * 128       # cols per (t, k) weight chunk
            # Per-(t,k) weight tiles: dep tracking is tile-granular and DMA
            # transfers serialize on the ring, so fine chunks let the first
            # matmuls start as soon as their own weights and x tile land.
            # Emission order (x tiles first, then weight chunks) is arranged
            # in the x-load loop below via _load_weight_chunk.
            if _MM_DTYPE == "bf16":
                bmtk = {
                    (t, k): wp.tile(
                        [128, perk], bdt, tag=f"bmt{t}_{k}", name=f"bmt{t}_{k}"
                    )
                    for t in range(T_ITERS)
                    for k in range(KCH)
                }

                def _load_weight_chunk(t, k):
                    off = t * pert + k * perk
                    nc.sync.dma_start(
                        out=bmtk[t, k][:], in_=bm[:, off : off + perk]
                    )
            else:
                bmt = wp.tile([128, bmw], bdt, tag="bmt")
                nc.sync.dma_start(out=bmt[:, 0:pert], in_=bm[:, 0:pert])

                def _load_weight_chunk(t, k):
                    if t == 0:
                        return  # loaded up front
                    if k == 0:
                        nc.sync.dma_start(
                            out=bmt[:, t * pert : (t + 1) * pert],
                            in_=bm[:, t * pert : (t + 1) * pert],
                        )
                    if t == 1 and k == 0 and bmw > T_ITERS * pert:
                        nc.sync.dma_start(
                            out=bmt[:, T_ITERS * pert :], in_=bm[:, T_ITERS * pert :]
                        )

            if _MM_DTYPE == "bf16":
                auxt = wp.tile([128, 16], F32, tag="auxt")  # DMA'd after im0 x

            _per_t = KCH * 13  # weight slots per iteration

            def bmat(idx):
                if _MM_DTYPE == "bf16":
                    t, r = divmod(idx, _per_t)
                    k, c = divmod(r, 13)
                    return bmtk[t, k][:, c * 128 : (c + 1) * 128]
                ap = bmt[:, idx * 128 : (idx + 1) * 128]
                if _MM_DTYPE == "f32r":
                    ap = ap.bitcast(F32R)
                return ap

            def mm_rhs(ap):
                if _MM_DTYPE == "f32r":
                    return ap.bitcast(F32R)
                return ap

            tset = _TILE_SET if _TILE_SET is not None else list(range(NTILES))
            xt = {}
            for im in range(IMGS):
                for j in tset:
                    tile = xp.tile([128, W_IMG], F32, tag=f"x{im}_{j}", name=f"x{im}_{j}")
                    xt[im, j] = tile
                    r0 = TSTART[j]
                    plo = max(0, -r0)
                    phi = min(128, H - r0)
                    if plo > 0 and phi == 128:
                        # memset only the pad rows: disjoint from the DMA's
                        # partition range, so the load isn't WAW-gated on it
                        nc.vector.memset(tile[0:plo, :], 0.0)
                    elif plo > 0 or phi < 128:
                        nc.vector.memset(tile[:], 0.0)
                    nc.sync.dma_start(
                        out=tile[plo:phi, :], in_=xs[im, r0 + plo : r0 + phi, :]
                    )
                    if im == 0 and j < KCH:
                        _load_weight_chunk(0, j)
                if im == 0 and _MM_DTYPE == "bf16":
                    nc.sync.dma_start(out=auxt[:], in_=aux[:])
                if 1 <= im < T_ITERS:
                    for k in range(KCH):
                        _load_weight_chunk(im, k)
                if im == 0:
                    # bias memsets after image-0 loads: emitted earlier they
                    # gate the first x DMA behind 12 vector-queue memsets
                    if _SIMPLE_BIAS:
                        bias_tiles = {}
                        for t in range(T_ITERS):
                            for k in range(KCH):
                                bb = wp.tile([128, 1], F32, tag=f"bias{t}_{k}", name=f"bias{t}_{k}")
                                nc.vector.memset(bb[:], float(bvals[t, k]))
                                bias_tiles[t, k] = bb
                    else:
                        bias_t = wp.tile([128, T_ITERS * KCH], F32, tag="bias")
                        for t in range(T_ITERS):
                            for k in range(KCH):
                                nc.vector.memset(
                                    bias_t[:, t * KCH + k : t * KCH + k + 1],
                                    float(bvals[t, k]),
                                )

            from concourse.ap import AP as _AP

            for it in range(T_ITERS):
                for im in range(IMGS):
                    corr_sb = None
                    if _MM_DTYPE == "bf16" and _BATCH_CORR:
                        # one bf16 copy of the whole image (5 tiles side by side)
                        xmi = sp.tile([128, NTILES * W_IMG], BF16, tag="xmi")
                        for j in tset:
                            nc.scalar.copy(
                                xmi[:, j * W_IMG : (j + 1) * W_IMG], xt[im, j][:]
                            )
                        if not _SKIP_CORR:
                            # batched edge corrections: [128,5] per (k,side)
                            corr_ps = ppc.tile([128, 8 * NTILES], F32, tag="corr")
                            xh = xmi[:].tensor
                            xoff = xmi[:].offset
                            pitch = NTILES * W_IMG
                            for k in range(KCH):
                                for side, skey, col in (
                                    (0, "corrL", 0),
                                    (1, "corrR", W_IMG - 1),
                                ):
                                    rhs = _AP(
                                        xh, xoff + col,
                                        [[pitch, 128], [W_IMG, NTILES]],
                                    )
                                    nc.tensor.matmul(
                                        corr_ps[
                                            :,
                                            (2 * k + side) * NTILES
                                            : (2 * k + side + 1) * NTILES,
                                        ],
                                        bmat(bindex[(it, k, skey)]),
                                        rhs,
                                        start=True,
                                        stop=True,
                                        skip_group_check=True,
                                    )
                            corr_sb = sp.tile([128, 8 * NTILES], F32, tag="corrsb")
                            nc.scalar.copy(corr_sb[:], corr_ps[:])
                    for j in tset:
                        x_t = xt[im, j]
                        cls = "top" if j == 0 else ("bot" if j == NTILES - 1 else "mid")
                        if _MM_DTYPE == "bf16":
                            if _BATCH_CORR:
                                xmm = xmi[:, j * W_IMG : (j + 1) * W_IMG]
                            else:
                                xmmt = sp.tile([128, W_IMG], BF16, tag="xb")
                                nc.scalar.copy(xmmt[:], x_t[:])
                                xmm = xmmt[:]
                        else:
                            xmm = x_t[:]
                        dks = []
                        for k in range(KCH):
                            dpool = pp1 if (_BATCH_CORR and k == 3) else pp
                            dk_t = dpool.tile([128, W_IMG], F32, tag=f"d{k}", name=f"d{k}")
                            dks.append(dk_t)
                        last_dx = DXS[-1]
                        for k in range(KCH):
                            base = 0
                            d = dks[k]
                            for Dx in DXS:
                                key = (
                                    (it, k, Dx, cls)
                                    if (it, k, Dx, cls) in bindex
                                    else (it, k, Dx, "mid")
                                )
                                ocl = max(0, -Dx)
                                och = W_IMG - max(0, Dx)
                                nc.tensor.matmul(
                                    d[:, base + ocl : base + och],
                                    bmat(bindex[key]),
                                    mm_rhs(xmm[:, ocl + Dx : och + Dx]),
                                    start=(Dx == 0),
                                    stop=(_BATCH_CORR and Dx == last_dx),
                                )
                            if _SKIP_CORR or _BATCH_CORR:
                                pass
                            else:
                                nc.tensor.matmul(
                                    d[:, base : base + 1],
                                    bmat(bindex[(it, k, "corrL")]),
                                    mm_rhs(xmm[:, 0:1]),
                                    start=False,
                                    stop=False,
                                )
                                nc.tensor.matmul(
                                    d[:, base + W_IMG - 1 : base + W_IMG],
                                    bmat(bindex[(it, k, "corrR")]),
                                    mm_rhs(xmm[:, W_IMG - 1 : W_IMG]),
                                    start=False,
                                    stop=True,
                                )
                        g = sp.tile([128, KCH * W_IMG], F32, tag="g")
                        # last image: sums ride DVE's in-order queue, so emit
                        # each partial sum right after its inputs' STTs
                        last_img = it == T_ITERS - 1 and im == IMGS - 1
                        sum_eng = nc.vector if last_img else nc.gpsimd
                        s01 = sp.tile([128, W_IMG], F32, tag="s01")
                        s23 = sp.tile([128, W_IMG], F32, tag="s23")
                        stot = sp.tile([128, W_IMG], F32, tag="stot")

                        def emit_s01():
                            sum_eng.tensor_tensor(
                                out=s01[:], in0=g[:, 0:512], in1=g[:, 512:1024],
                                op=ALU.add,
                            )

                        if _MM_DTYPE == "bf16":
                            mask_ap = auxt[:, j : j + 1] if _MASK_AP else -C_UPD
                        else:
                            mask_ap = (
                                bmt[:, _NB * 128 + j : _NB * 128 + j + 1]
                                if _MASK_AP
                                else -C_UPD
                            )

                        def emit_s23():
                            sum_eng.tensor_tensor(
                                out=s23[:], in0=g[:, 1024:1536], in1=g[:, 1536:2048],
                                op=ALU.add,
                            )

                        for k in range(KCH):
                            base = k * W_IMG
                            if corr_sb is not None and not _SKIP_CORR:
                                # add the two edge-correction columns into d
                                dh = dks[k][:].tensor
                                doff = dks[k][:].offset
                                dap = _AP(dh, doff, [[W_IMG, 128], [W_IMG - 1, 2]])
                                ch = corr_sb[:].tensor
                                coff = corr_sb[:].offset
                                cap = _AP(
                                    ch, coff + 2 * k * NTILES + j,
                                    [[8 * NTILES, 128], [NTILES, 2]],
                                )
                                nc.vector.tensor_tensor(
                                    out=dap, in0=dap, in1=cap, op=ALU.add
                                )
                            ek = sp.tile([128, W_IMG], F32, tag=f"e{k}")
                            nc.scalar.activation(
                                ek[:],
                                dks[k][:],
                                AF.Derivative_Erf,
                                bias=(bias_tiles[it, k][:, 0:1] if _SIMPLE_BIAS
                                      else bias_t[:, it * KCH + k : it * KCH + k + 1]),
                                scale=1.0,
                            )
                            nc.vector.scalar_tensor_tensor(
                                out=g[:, base : base + W_IMG],
                                in0=dks[k][:],
                                scalar=float(bvals[it, k]),
                                in1=ek[:],
                                op0=ALU.add,
                                op1=ALU.mult,
                            )
                            if last_img and k == 1:
                                emit_s01()
                            if last_img and k == 3:
                                emit_s23()
                        if not last_img:
                            emit_s01()
                            emit_s23()
                        sum_eng.tensor_tensor(
                            out=stot[:], in0=s01[:], in1=s23[:], op=ALU.add
                        )
                        if _INPLACE_UPD:
                            nc.vector.scalar_tensor_tensor(
                                out=x_t[:],
                                in0=stot[:],
                                scalar=mask_ap,
                                in1=x_t[:],
                                op0=ALU.mult,
                                op1=ALU.add,
                            )
                        else:
                            x_new = xp.tile([128, W_IMG], F32, tag=f"xn{im}_{j}_{it}")
                            nc.vector.scalar_tensor_tensor(
                                out=x_new[:],
                                in0=stot[:],
                                scalar=mask_ap,
                                in1=x_t[:],
                                op0=ALU.mult,
                                op1=ALU.add,
                            )
                            xt[im, j] = x_new

            for im in range(IMGS):
                for j in tset:
                    rows = CORE_ROWS[j]
                    nc.sync.dma_start(
                        out=yo[im, 116 * j : 116 * j + rows, :],
                        in_=xt[im, j][CORE_LO : CORE_LO + rows, :],
                    )
    nc.compile()
    return nc


_CACHE = {}


def _get_program(Wc, bc):
    key = (Wc.tobytes(), bc.tobytes())
    if key not in _CACHE:
        barr, bindex = _build_bmats(Wc.astype(np.float64))
        # SBUF layout [p, n*128+m]
        if _MM_DTYPE == "bf16":
            parts = [barr.transpose(1, 0, 2).reshape(128, _NB * 128)]
            w0 = _NB * 128
        else:
            parts = [barr.transpose(1, 0, 2).reshape(128, _NB * 128), _build_masks()]
            w0 = _NB * 128 + NTILES
        if _PAD_BMAT:
            wpad = (w0 + 127) // 128 * 128 - w0
            if wpad:
                parts.append(np.zeros((128, wpad), np.float32))
        bflat = np.ascontiguousarray(np.concatenate(parts, axis=1), dtype=np.float32)
        if _MM_DTYPE == "bf16":
            import ml_dtypes

            bflat = bflat.astype(ml_dtypes.bfloat16)
        nc = _build_nc(bc.astype(np.float64), bindex)
        _CACHE[key] = (nc, bflat)
    return _CACHE[key]


def _install_trace_shim():
    """The agent image lacks antenv.axon_hooks; rebuild the NTFF hook from
    trn_boot's ctypes recipe and skip the artifact upload."""
    import types

    if "antenv.axon_hooks" in sys.modules:
        return
    try:
        from trn_agent_boot.trn_boot import _ntff_profile_via_ctypes

        hook = _ntff_profile_via_ctypes("/opt/axon/libaxon_pjrt.so")
    except Exception:
        hook = None
    mod = types.ModuleType("antenv.axon_hooks")
    mod.get_axon_ntff_profile_hook = lambda: hook
    mod.set_axon_ntff_profile_hook = lambda h: None
    sys.modules["antenv.axon_hooks"] = mod
    import concourse.bass_utils as bu

    bu.upload_artifacts = lambda d: "local://skipped"


def kernel(x, W, b, _trace=False, _tracedir=None):
    x = np.asarray(x)
    W = np.asarray(W)
    b = np.asarray(b)
    nc, bflat = _get_program(W, b)
    in_maps = []
    for c in range(NCORES):
        shard = np.ascontiguousarray(x[c * IMGS : (c + 1) * IMGS, 0]).astype(np.float32)
        im_map = {"xs": shard, "bmat": bflat}
        if _MM_DTYPE == "bf16":
            am = np.zeros((128, 16), np.float32)
            am[:, :NTILES] = _build_masks()
            im_map["aux"] = am
        in_maps.append(im_map)
    kw = {}
    if _trace:
        _install_trace_shim()
        kw = {"trace": True, "tmpdir": _tracedir}
    res = run_bass_kernel_spmd(nc, in_maps, list(range(NCORES)), **kw)
    out = np.concatenate([res.results[c]["out"] for c in range(NCORES)], axis=0)
    out = out[:, None].astype(x.dtype)
    kernel._last = res
    return out



# revision 25
# speedup vs baseline: 1.2581x; 1.0001x over previous
"""Trainium2 Bass kernel for nn_Deep_AD_F_58213986730479 (dense_cnn).

Math (per iteration t of 3):
    feats = 4 one-pixel zero-padded shifts (N,S,W,E) of x        [n,4,h,w]
    d     = conv3x3(feats, W[t]) + b[t]                          [n,4,h,w]
    x    -= sum_k d_k * exp(-d_k^2) / 4

Implementation:
  - Pure data parallel: batch 32 -> 8 cores x 4 images.
  - The shift+conv composes into a 21-tap stencil on x. Vertical taps are
    applied with banded-matrix matmuls on TensorE (contraction over image
    rows on partitions); horizontal taps via 5 column-shifted accumulating
    matmuls into PSUM. Boundary semantics of the double zero-padding are
    exact: row-edge terms fold into per-tile band-matrix variants; column
    edge terms are two N=1 correction matmuls per channel.
  - Matmuls default to bf16 (1 cycle/row on the PE vs 4 for fp32); x stays
    f32 in SBUF and is converted per-tile by a ScalarE copy each iteration,
    so only the stencil inputs are rounded (rel err ~7e-4, gate is 2e-2).
  - exp(-d^2) comes from one ScalarE op: Derivative_Erf = 2/sqrt(pi)*exp(-x^2);
    the bias add (d+b) rides free in the activation and in the DVE
    scalar_tensor_tensor that forms gated = (d+b)*e. Channel sum on GpSimd,
    final x update is one fused DVE scalar_tensor_tensor.
  - Each 512x512 image is 5 row-tiles [128,512] (stride 116, 6-row halo);
    3 iterations shrink the valid halo by 2 rows each, so no cross-tile
    traffic is ever needed.
  - Startup critical path: weights are split into per-(t,k) SBUF tiles
    (dep tracking is tile-granular) and their DMAs are interleaved after
    image-0's x tiles so the first matmuls start ~10us earlier.
"""
import sys

sys.path.insert(0, "/opt/trn_rl_repo")

import math
import numpy as np

import concourse.bass as bass
import concourse.bacc as bacc
import concourse.mybir as mybir
from concourse.tile import TileContext
from concourse.bass_utils import run_bass_kernel_spmd

F32 = mybir.dt.float32
F32R = mybir.dt.float32r
BF16 = mybir.dt.bfloat16
AF = mybir.ActivationFunctionType
ALU = mybir.AluOpType

NCORES = 8
IMGS = 4          # images per core
H = W_IMG = 512
T_ITERS = 3
KCH = 4
NTILES = 5
TSTART = [-6, 110, 226, 342, 458]   # image row held by partition 0 of tile j
CORE_LO = 6                          # first owned partition of each tile
CORE_ROWS = [116, 116, 116, 116, 48]
C_UPD = math.sqrt(math.pi) / 8.0     # 1/4 * sqrt(pi)/2 (Derivative_Erf scale)

# feats channel order in reference: N, S, W, E
OY = [-1, 1, 0, 0]
OX = [0, 0, -1, 1]

DXS = [0, -1, 1, -2, 2]

# debug bisect flags
_SKIP_CORR = False
_BATCH_CORR = False  # batch edge-corrections: 5-col matmuls per (t,im,k,side)
_PSUM_BUFS = 2
_INPLACE_UPD = True
_MASK_AP = True
_TILE_SET = None  # e.g. [2] to restrict tiles (debug)
_PAD_BMAT = True
_MM_DTYPE = __import__("os").environ.get("KERNEL_MM_DTYPE", "bf16")  # f32 | f32r | bf16
_BATCH_CORR = _BATCH_CORR and _MM_DTYPE == "bf16"
_SIMPLE_BIAS = False  # Dx=0 first: full-range start=True write


def _composite_taps(Wc):
    """T[t,k,Dy+2,Dx+2] = sum of W[t,k,i,dy+1,dx+1] with dy+oy_i=Dy, dx+ox_i=Dx."""
    taps = np.zeros((T_ITERS, KCH, 5, 5), np.float64)
    for t in range(T_ITERS):
        for k in range(KCH):
            for i in range(4):
                for dy in (-1, 0, 1):
                    for dx in (-1, 0, 1):
                        taps[t, k, dy + OY[i] + 2, dx + OX[i] + 2] += Wc[
                            t, k, i, dy + 1, dx + 1
                        ]
    return taps


def _build_bmats(Wc):
    """Dense lhsT matrices, returned as array [NB,128,128] f32 plus an index fn.

    Layout per (t,k): 5 generic B_Dx, then 3 top-variant (Dx=-1,0,1), then
    3 bottom-variant, then left corr, right corr = 13 matrices.
    B[in_row, out_row] = tap[in-out, Dx].
    """
    taps = _composite_taps(Wc)
    mats = []
    index = {}

    def band(vals_by_dy):
        B = np.zeros((128, 128), np.float64)
        for dy, v in vals_by_dy.items():
            B += v * np.eye(128, k=-dy)
        return B

    for t in range(T_ITERS):
        for k in range(KCH):
            per_dx = {}
            for Dx in (-2, -1, 0, 1, 2):
                per_dx[Dx] = band(
                    {Dy: taps[t, k, Dy + 2, Dx + 2] for Dy in range(-2, 3)}
                )
            for Dx in DXS:
                index[(t, k, Dx, "mid")] = len(mats)
                mats.append(per_dx[Dx])
            for Dx in (-1, 0, 1):
                Btop = per_dx[Dx].copy()
                # image row 0 = partition CORE_LO of tile 0: remove south-ch dy=-1
                Btop[CORE_LO, CORE_LO] -= Wc[t, k, 1, 0, Dx + 1]
                index[(t, k, Dx, "top")] = len(mats)
                mats.append(Btop)
            for Dx in (-1, 0, 1):
                Bbot = per_dx[Dx].copy()
                # image row 511 = partition 53 of tile 4: remove north-ch dy=+1
                p = CORE_LO + (H - 1) - TSTART[4] - CORE_LO  # = 53
                Bbot[p, p] -= Wc[t, k, 0, 2, Dx + 1]
                index[(t, k, Dx, "bot")] = len(mats)
                mats.append(Bbot)
            # column-edge corrections (vertical 3-tap bands)
            Bl = band({dy: -Wc[t, k, 3, dy + 1, 0] for dy in (-1, 0, 1)})
            index[(t, k, "corrL")] = len(mats)
            mats.append(Bl)
            Br = band({dy: -Wc[t, k, 2, dy + 1, 2] for dy in (-1, 0, 1)})
            index[(t, k, "corrR")] = len(mats)
            mats.append(Br)
    arr = np.stack(mats).astype(np.float32)
    return arr, index


_NB = T_ITERS * KCH * 13


def _build_masks():
    """Per-tile update masks [128, NTILES]: -C_UPD at real image rows, 0 at pad."""
    m = np.full((128, NTILES), -C_UPD, np.float32)
    for j in range(NTILES):
        r0 = TSTART[j]
        plo = max(0, -r0)
        phi = min(128, H - r0)
        m[0:plo, j] = 0.0
        m[phi:128, j] = 0.0
    return m


def _build_nc(bvals, bindex):
    nc = bacc.Bacc(None, target_bir_lowering=False)
    xs = nc.declare_dram_parameter("xs", [IMGS, H, W_IMG], F32, isOutput=False)
    bmw = _NB * 128 + (0 if _MM_DTYPE == "bf16" else NTILES)
    if _PAD_BMAT:
        bmw = (bmw + 127) // 128 * 128
    bdt = BF16 if _MM_DTYPE == "bf16" else F32
    bm = nc.declare_dram_parameter("bmat", [128, bmw], bdt, isOutput=False)
    if _MM_DTYPE == "bf16":
        aux = nc.declare_dram_parameter("aux", [128, 16], F32, isOutput=False)
    yo = nc.declare_dram_parameter("out", [IMGS, H, W_IMG], F32, isOutput=True)

    with TileContext(nc) as tc:
        with (
            tc.tile_pool(name="wts", bufs=1) as wp,
            tc.tile_pool(name="xdata", bufs=1) as xp,
            tc.tile_pool(name="work", bufs=4) as sp,
            tc.tile_pool(name="ps", bufs=2, space="PSUM") as pp,
            tc.tile_pool(name="ps1", bufs=1, space="PSUM") as pp1,
            tc.tile_pool(name="psc", bufs=1, space="PSUM") as ppc,
        ):
            pert = KCH * 13 * 128  # cols per iteration t
            perk = 13 * 128       # cols per (t, k) weight chunk
            # Per-(t,k) weight tiles: dep tracking is tile-granular and DMA
            # transfers serialize on the ring, so fine chunks let the first
            # matmuls start as soon as their own weights and x tile land.
            # Emission order (x tiles first, then weight chunks) is arranged
            # in the x-load loop below via _load_weight_chunk.
            if _MM_DTYPE == "bf16":
                bmtk = {
                    (t, k): wp.tile(
                        [128, perk], bdt, tag=f"bmt{t}_{k}", name=f"bmt{t}_{k}"
                    )
                    for t in range(T_ITERS)
                    for k in range(KCH)
                }

                def _load_weight_chunk(t, k):
                    off = t * pert + k * perk
                    nc.sync.dma_start(
                        out=bmtk[t, k][:], in_=bm[:, off : off + perk]
                    )
            else:
                bmt = wp.tile([128, bmw], bdt, tag="bmt")
                nc.sync.dma_start(out=bmt[:, 0:pert], in_=bm[:, 0:pert])

                def _load_weight_chunk(t, k):
                    if t == 0:
                        return  # loaded up front
                    if k == 0:
                        nc.sync.dma_start(
                            out=bmt[:, t * pert : (t + 1) * pert],
                            in_=bm[:, t * pert : (t + 1) * pert],
                        )
                    if t == 1 and k == 0 and bmw > T_ITERS * pert:
                        nc.sync.dma_start(
                            out=bmt[:, T_ITERS * pert :], in_=bm[:, T_ITERS * pert :]
                        )

            if _MM_DTYPE == "bf16":
                auxt = wp.tile([128, 16], F32, tag="auxt")  # DMA'd after im0 x

            _per_t = KCH * 13  # weight slots per iteration

            def bmat(idx):
                if _MM_DTYPE == "bf16":
                    t, r = divmod(idx, _per_t)
                    k, c = divmod(r, 13)
                    return bmtk[t, k][:, c * 128 : (c + 1) * 128]
                ap = bmt[:, idx * 128 : (idx + 1) * 128]
                if _MM_DTYPE == "f32r":
                    ap = ap.bitcast(F32R)
                return ap

            def mm_rhs(ap):
                if _MM_DTYPE == "f32r":
                    return ap.bitcast(F32R)
                return ap

            tset = _TILE_SET if _TILE_SET is not None else list(range(NTILES))
            xt = {}
            for im in range(IMGS):
                for j in tset:
                    tile = xp.tile([128, W_IMG], F32, tag=f"x{im}_{j}", name=f"x{im}_{j}")
                    xt[im, j] = tile
                    r0 = TSTART[j]
                    plo = max(0, -r0)
                    phi = min(128, H - r0)
                    if plo > 0 and phi == 128:
                        # memset only the pad rows: disjoint from the DMA's
                        # partition range, so the load isn't WAW-gated on it
                        nc.vector.memset(tile[0:plo, :], 0.0)
                    elif plo > 0 or phi < 128:
                        nc.vector.memset(tile[:], 0.0)
                    nc.sync.dma_start(
                        out=tile[plo:phi, :], in_=xs[im, r0 + plo : r0 + phi, :]
                    )
                    if im == 0 and j < KCH:
                        _load_weight_chunk(0, j)
                if im == 0 and _MM_DTYPE == "bf16":
                    nc.sync.dma_start(out=auxt[:], in_=aux[:])
                if 1 <= im < T_ITERS:
                    for k in range(KCH):
                        _load_weight_chunk(im, k)
                if im == 0:
                    # bias memsets after image-0 loads: emitted earlier they
                    # gate the first x DMA behind 12 vector-queue memsets
                    if _SIMPLE_BIAS:
                        bias_tiles = {}
                        for t in range(T_ITERS):
                            for k in range(KCH):
                                bb = wp.tile([128, 1], F32, tag=f"bias{t}_{k}", name=f"bias{t}_{k}")
                                nc.vector.memset(bb[:], float(bvals[t, k]))
                                bias_tiles[t, k] = bb
                    else:
                        bias_t = wp.tile([128, T_ITERS * KCH], F32, tag="bias")
                        for t in range(T_ITERS):
                            for k in range(KCH):
                                nc.vector.memset(
                                    bias_t[:, t * KCH + k : t * KCH + k + 1],
                                    float(bvals[t, k]),
                                )

            from concourse.ap import AP as _AP

            for it in range(T_ITERS):
                for im in range(IMGS):
                    corr_sb = None
                    if _MM_DTYPE == "bf16" and _BATCH_CORR:
                        # one bf16 copy of the whole image (5 tiles side by side)
                        xmi = sp.tile([128, NTILES * W_IMG], BF16, tag="xmi")
                        for j in tset:
                            nc.scalar.copy(
                                xmi[:, j * W_IMG : (j + 1) * W_IMG], xt[im, j][:]
                            )
                        if not _SKIP_CORR:
                            # batched edge corrections: [128,5] per (k,side)
                            corr_ps = ppc.tile([128, 8 * NTILES], F32, tag="corr")
                            xh = xmi[:].tensor
                            xoff = xmi[:].offset
                            pitch = NTILES * W_IMG
                            for k in range(KCH):
                                for side, skey, col in (
                                    (0, "corrL", 0),
                                    (1, "corrR", W_IMG - 1),
                                ):
                                    rhs = _AP(
                                        xh, xoff + col,
                                        [[pitch, 128], [W_IMG, NTILES]],
                                    )
                                    nc.tensor.matmul(
                                        corr_ps[
                                            :,
                                            (2 * k + side) * NTILES
                                            : (2 * k + side + 1) * NTILES,
                                        ],
                                        bmat(bindex[(it, k, skey)]),
                                        rhs,
                                        start=True,
                                        stop=True,
                                        skip_group_check=True,
                                    )
                            corr_sb = sp.tile([128, 8 * NTILES], F32, tag="corrsb")
                            nc.scalar.copy(corr_sb[:], corr_ps[:])
                    for j in tset:
                        x_t = xt[im, j]
                        cls = "top" if j == 0 else ("bot" if j == NTILES - 1 else "mid")
                        if _MM_DTYPE == "bf16":
                            if _BATCH_CORR:
                                xmm = xmi[:, j * W_IMG : (j + 1) * W_IMG]
                            else:
                                xmmt = sp.tile([128, W_IMG], BF16, tag="xb")
                                nc.scalar.copy(xmmt[:], x_t[:])
                                xmm = xmmt[:]
                        else:
                            xmm = x_t[:]
                        dks = []
                        for k in range(KCH):
                            dpool = pp1 if (_BATCH_CORR and k == 3) else pp
                            dk_t = dpool.tile([128, W_IMG], F32, tag=f"d{k}", name=f"d{k}")
                            dks.append(dk_t)
                        last_dx = DXS[-1]
                        for k in range(KCH):
                            base = 0
                            d = dks[k]
                            for Dx in DXS:
                                key = (
                                    (it, k, Dx, cls)
                                    if (it, k, Dx, cls) in bindex
                                    else (it, k, Dx, "mid")
                                )
                                ocl = max(0, -Dx)
                                och = W_IMG - max(0, Dx)
                                nc.tensor.matmul(
                                    d[:, base + ocl : base + och],
                                    bmat(bindex[key]),
                                    mm_rhs(xmm[:, ocl + Dx : och + Dx]),
                                    start=(Dx == 0),
                                    stop=(_BATCH_CORR and Dx == last_dx),
                                )
                            if _SKIP_CORR or _BATCH_CORR:
                                pass
                            else:
                                nc.tensor.matmul(
                                    d[:, base : base + 1],
                                    bmat(bindex[(it, k, "corrL")]),
                                    mm_rhs(xmm[:, 0:1]),
                                    start=False,
                                    stop=False,
                                )
                                nc.tensor.matmul(
                                    d[:, base + W_IMG - 1 : base + W_IMG],
                                    bmat(bindex[(it, k, "corrR")]),
                                    mm_rhs(xmm[:, W_IMG - 1 : W_IMG]),
                                    start=False,
                                    stop=True,
                                )
                        g = sp.tile([128, KCH * W_IMG], F32, tag="g")
                        # last image: sums ride DVE's in-order queue, so emit
                        # each partial sum right after its inputs' STTs
                        last_img = it == T_ITERS - 1 and im == IMGS - 1
                        sum_eng = nc.vector if last_img else nc.gpsimd
                        s01 = sp.tile([128, W_IMG], F32, tag="s01")
                        s23 = sp.tile([128, W_IMG], F32, tag="s23")
                        stot = sp.tile([128, W_IMG], F32, tag="stot")

                        def emit_s01():
                            sum_eng.tensor_tensor(
                                out=s01[:], in0=g[:, 0:512], in1=g[:, 512:1024],
                                op=ALU.add,
                            )

                        if _MM_DTYPE == "bf16":
                            mask_ap = auxt[:, j : j + 1] if _MASK_AP else -C_UPD
                        else:
                            mask_ap = (
                                bmt[:, _NB * 128 + j : _NB * 128 + j + 1]
                                if _MASK_AP
                                else -C_UPD
                            )

                        def emit_s23():
                            sum_eng.tensor_tensor(
                                out=s23[:], in0=g[:, 1024:1536], in1=g[:, 1536:2048],
                                op=ALU.add,
                            )

                        for k in range(KCH):
                            base = k * W_IMG
                            if corr_sb is not None and not _SKIP_CORR:
                                # add the two edge-correction columns into d
                                dh = dks[k][:].tensor
                                doff = dks[k][:].offset
                                dap = _AP(dh, doff, [[W_IMG, 128], [W_IMG - 1, 2]])
                                ch = corr_sb[:].tensor
                                coff = corr_sb[:].offset
                                cap = _AP(
                                    ch, coff + 2 * k * NTILES + j,
                                    [[8 * NTILES, 128], [NTILES, 2]],
                                )
                                nc.vector.tensor_tensor(
                                    out=dap, in0=dap, in1=cap, op=ALU.add
                                )
                            ek = sp.tile([128, W_IMG], F32, tag=f"e{k}")
                            nc.scalar.activation(
                                ek[:],
                                dks[k][:],
                                AF.Derivative_Erf,
                                bias=(bias_tiles[it, k][:, 0:1] if _SIMPLE_BIAS
                                      else bias_t[:, it * KCH + k : it * KCH + k + 1]),
                                scale=1.0,
                            )
                            nc.vector.scalar_tensor_tensor(
                                out=g[:, base : base + W_IMG],
                                in0=dks[k][:],
                                scalar=float(bvals[it, k]),
                                in1=ek[:],
                                op0=ALU.add,
                                op1=ALU.mult,
                            )
                            if last_img and k == 1:
                                emit_s01()
                            if last_img and k == 3:
                                emit_s23()
                        if not last_img:
                            emit_s01()
                            emit_s23()
                        sum_eng.tensor_tensor(
                            out=stot[:], in0=s01[:], in1=s23[:], op=ALU.add
                        )
                        if _INPLACE_UPD:
                            nc.vector.scalar_tensor_tensor(
                                out=x_t[:],
                                in0=stot[:],
                                scalar=mask_ap,
                                in1=x_t[:],
                                op0=ALU.mult,
                                op1=ALU.add,
                            )
                        else:
                            x_new = xp.tile([128, W_IMG], F32, tag=f"xn{im}_{j}_{it}")
                            nc.vector.scalar_tensor_tensor(
                                out=x_new[:],
                                in0=stot[:],
                                scalar=mask_ap,
                                in1=x_t[:],
                                op0=ALU.mult,
                                op1=ALU.add,
                            )
                            xt[im, j] = x_new

            for im in range(IMGS):
                for j in tset:
                    rows = CORE_ROWS[j]
                    nc.sync.dma_start(
                        out=yo[im, 116 * j : 116 * j + rows, :],
                        in_=xt[im, j][CORE_LO : CORE_LO + rows, :],
                    )
    nc.compile()
    return nc


_CACHE = {}


def _get_program(Wc, bc):
    key = (Wc.tobytes(), bc.tobytes())
    if key not in _CACHE:
        barr, bindex = _build_bmats(Wc.astype(np.float64))
        # SBUF layout [p, n*128+m]
        if _MM_DTYPE == "bf16":
            parts = [barr.transpose(1, 0, 2).reshape(128, _NB * 128)]
            w0 = _NB * 128
        else:
            parts = [barr.transpose(1, 0, 2).reshape(128, _NB * 128), _build_masks()]
            w0 = _NB * 128 + NTILES
        if _PAD_BMAT:
            wpad = (w0 + 127) // 128 * 128 - w0
            if wpad:
                parts.append(np.zeros((128, wpad), np.float32))
        bflat = np.ascontiguousarray(np.concatenate(parts, axis=1), dtype=np.float32)
        if _MM_DTYPE == "bf16":
            import ml_dtypes

            bflat = bflat.astype(ml_dtypes.bfloat16)
        nc = _build_nc(bc.astype(np.float64), bindex)
        _CACHE[key] = (nc, bflat)
    return _CACHE[key]


def _install_trace_shim():
    """The agent image lacks antenv.axon_hooks; rebuild the NTFF hook from
    trn_boot's ctypes recipe and skip the artifact upload."""
    import types

    if "antenv.axon_hooks" in sys.modules:
        return
    try:
        from trn_agent_boot.trn_boot import _ntff_profile_via_ctypes

        hook = _ntff_profile_via_ctypes("/opt/axon/libaxon_pjrt.so")
    except Exception:
        hook = None
    mod = types.ModuleType("antenv.axon_hooks")
    mod.get_axon_ntff_profile_hook = lambda: hook
    mod.set_axon_ntff_profile_hook = lambda h: None
    sys.modules["antenv.axon_hooks"] = mod
    import concourse.bass_utils as bu

    bu.upload_artifacts = lambda d: "local://skipped"


def kernel(x, W, b, _trace=False, _tracedir=None):
    x = np.asarray(x)
    W = np.asarray(W)
    b = np.asarray(b)
    nc, bflat = _get_program(W, b)
    in_maps = []
    for c in range(NCORES):
        shard = np.ascontiguousarray(x[c * IMGS : (c + 1) * IMGS, 0]).astype(np.float32)
        im_map = {"xs": shard, "bmat": bflat}
        if _MM_DTYPE == "bf16":
            am = np.zeros((128, 16), np.float32)
            am[:, :NTILES] = _build_masks()
            im_map["aux"] = am
        in_maps.append(im_map)
    kw = {}
    if _trace:
        _install_trace_shim()
        kw = {"trace": True, "tmpdir": _tracedir}
    res = run_bass_kernel_spmd(nc, in_maps, list(range(NCORES)), **kw)
    out = np.concatenate([res.results[c]["out"] for c in range(NCORES)], axis=0)
    out = out[:, None].astype(x.dtype)
    kernel._last = res
    return out

